# revision 2
# baseline (speedup 1.0000x reference)
"""Chunked cross-entropy loss on 8 TRN2 NeuronCores (Bass/Tile).

Strategy (vocab/tensor parallel):
  - weight_matrix [V=131072, D=2048] sharded along vocab across 8 cores
    (16384 rows each); hidden_states replicated (each core computes all
    N=8192 token logits for its vocab shard).
  - Per core: tiled bf16 matmul h @ Wc^T with fp32 PSUM accumulation.
    For every [128 tok x 512 voc] logits tile the device emits
    (-max, sum_exp(logit - max)) per token row, plus the target logit
    contribution (mask-select against the live PSUM tile: exactly one
    (core, tile) contains each token's target column; all others add 0).
  - Host: float64 logsumexp-merge of the 8*8*4 = 256 per-token partials per
    token, target logit = sum of contributions, loss = mean(lse - tgt).

Numerics: bf16 inputs / fp32 accumulation. Per-tile true max means no
fixed-shift overflow/underflow risk for any input distribution.
"""

import numpy as np
import ml_dtypes

import concourse.bass as bass
import concourse.mybir as mybir
import concourse.tile as tile
from concourse import bacc
from concourse.bass_utils import run_bass_kernel_spmd

# Problem shape (hardcoded per contract).
B, S, D, V = 4, 2048, 2048, 131072
N_TOK = B * S                  # 8192 tokens
NCORES = 8
P = 128                        # partitions
VSHARD = V // NCORES           # 16384 vocab rows per core
MMF = 512                      # matmul moving free dim (one PSUM bank fp32)

# Default tiling: vocab block 2048 (resident in SBUF), 4 PSUM banks per pair.
N_K = D // P                   # 16 contraction tiles
N_J = 4                        # 512-col logits tiles per vocab block
VB = N_J * MMF                 # 2048 vocab block
N_VB = VSHARD // VB            # 8 vocab blocks per core
N_T = N_TOK // P               # 64 token tiles

BF16 = ml_dtypes.bfloat16

_COMPILED = {}


def build_nc(n_t=N_T, n_k=N_K, n_vb=N_VB, n_j=N_J, num_devices=NCORES,
             w_bufs=2):
    """Build + compile the per-core Bass program (SPMD: same program on all
    cores, per-core data differs)."""
    vb = n_j * MMF
    d = n_k * P
    nc = bacc.Bacc("TRN2", target_bir_lowering=False, debug=False,
                   num_devices=num_devices)

    wt = nc.dram_tensor("wt", [P, n_vb, n_k, vb], mybir.dt.bfloat16,
                        kind="ExternalInput")
    ht = nc.dram_tensor("ht", [n_t, P, n_k, P], mybir.dt.bfloat16,
                        kind="ExternalInput")
    tg = nc.dram_tensor("tg", [P, n_vb, n_t], mybir.dt.float32,
                        kind="ExternalInput")
    io = nc.dram_tensor("io", [P, vb], mybir.dt.float32,
                        kind="ExternalInput")
    negm = nc.dram_tensor("negm", [P, n_vb, n_t, n_j], mybir.dt.float32,
                          kind="ExternalOutput")
    ssum = nc.dram_tensor("ssum", [P, n_vb, n_t, n_j], mybir.dt.float32,
                          kind="ExternalOutput")
    tgtv = nc.dram_tensor("tgtv", [P, n_vb, n_t, n_j], mybir.dt.float32,
                          kind="ExternalOutput")

    with tile.TileContext(nc) as tc:
        with (
            tc.tile_pool(name="wp", bufs=w_bufs) as wp,
            tc.tile_pool(name="hp", bufs=3) as hp,
            tc.tile_pool(name="pp", bufs=2, space=bass.MemorySpace.PSUM) as pp,
            tc.tile_pool(name="scr", bufs=2) as scr,
            tc.tile_pool(name="cst", bufs=1) as cst,
        ):
            iota_t = cst.tile([P, vb], mybir.dt.float32)
            nc.sync.dma_start(iota_t[:], io.ap())
            tg_t = cst.tile([P, n_vb, n_t], mybir.dt.float32)
            nc.sync.dma_start(tg_t[:], tg.ap())
            negm_t = cst.tile([P, n_vb, n_t, n_j], mybir.dt.float32)
            ssum_t = cst.tile([P, n_vb, n_t, n_j], mybir.dt.float32)
            tgtv_t = cst.tile([P, n_vb, n_t, n_j], mybir.dt.float32)

            for ivb in range(n_vb):
                # vocab-block weights stay resident for the whole token loop;
                # gpsimd (SWDGE) ring so the big load never head-of-line
                # blocks the token-tile loads on the sync (HWDGE) ring.
                w_t = wp.tile([P, n_k, vb], mybir.dt.bfloat16)
                nc.gpsimd.dma_start(w_t[:], wt.ap()[:, ivb])
                for t in range(n_t):
                    h_t = hp.tile([P, n_k, P], mybir.dt.bfloat16)
                    nc.sync.dma_start(h_t[:], ht.ap()[t])
                    ps = pp.tile([P, n_j, MMF], mybir.dt.float32)
                    for j in range(n_j):
                        for k in range(n_k):
                            nc.tensor.matmul(
                                ps[:, j, :],
                                h_t[:, k, :],
                                w_t[:, k, j * MMF:(j + 1) * MMF],
                                start=(k == 0),
                                stop=(k == n_k - 1),
                            )
                    # per-tile -max for all n_j tiles in one reduce
                    nm = negm_t[:, ivb, t, :]
                    nc.vector.reduce_max(nm, ps[:], axis=mybir.AxisListType.X,
                                         negate=True)
                    for j in range(n_j):
                        es = scr.tile([P, MMF], mybir.dt.float32)
                        nc.scalar.activation(
                            es[:], ps[:, j, :], mybir.ActivationFunctionType.Exp,
                            bias=negm_t[:, ivb, t, j:j + 1],
                            accum_out=ssum_t[:, ivb, t, j:j + 1])
                        # target logit hit: (iota == tgt_col) * logits, summed
                        mo = scr.tile([P, MMF], mybir.dt.float32)
                        nc.vector.scalar_tensor_tensor(
                            out=mo[:], in0=iota_t[:, j * MMF:(j + 1) * MMF],
                            scalar=tg_t[:, ivb, t:t + 1], in1=ps[:, j, :],
                            op0=mybir.AluOpType.is_equal,
                            op1=mybir.AluOpType.mult,
                            accum_out=tgtv_t[:, ivb, t, j:j + 1])

            nc.sync.dma_start(negm.ap(), negm_t[:])
            nc.sync.dma_start(ssum.ap(), ssum_t[:])
            nc.sync.dma_start(tgtv.ap(), tgtv_t[:])

    nc.compile()
    return nc


def _get_compiled():
    key = "full"
    if key not in _COMPILED:
        _COMPILED[key] = build_nc()
    return _COMPILED[key]


def _prep_inputs(hidden_states, targets, weight_matrix):
    """Host-side shard + layout prep. Returns per-core in_maps."""
    h = np.ascontiguousarray(np.asarray(hidden_states, dtype=np.float32)
                             ).reshape(N_TOK, D)
    tgt = np.asarray(targets).reshape(N_TOK).astype(np.int64)
    W = np.asarray(weight_matrix, dtype=np.float32)

    # h blocked: [t, p(d within k-tile), k, m(token within tile)]
    hb = np.ascontiguousarray(
        h.astype(BF16).reshape(N_T, P, N_K, P).transpose(0, 3, 2, 1))

    iota = np.ascontiguousarray(
        np.broadcast_to(np.arange(VB, dtype=np.float32), (P, VB)))

    tl = tgt.reshape(N_T, P)  # [t, p]
    vb_off = (np.arange(N_VB, dtype=np.int64) * VB)[None, :, None]

    in_maps = []
    for c in range(NCORES):
        Wc = W[c * VSHARD:(c + 1) * VSHARD]
        wb = np.ascontiguousarray(
            Wc.astype(BF16).reshape(N_VB, VB, N_K, P).transpose(3, 0, 2, 1))
        tgl = (tl.T[:, None, :] - c * VSHARD - vb_off).astype(np.float32)
        in_maps.append({"wt": wb, "ht": hb, "tg": np.ascontiguousarray(tgl),
                        "io": iota})
    return in_maps


def _combine(results):
    """float64 logsumexp-merge of per-core per-tile partials -> scalar loss."""
    m = np.stack([-r["negm"].astype(np.float64) for r in results])  # [C,P,vb,t,j]
    s = np.stack([r["ssum"].astype(np.float64) for r in results])
    tv = np.stack([r["tgtv"].astype(np.float64) for r in results])

    # partial axes: core, vb, j -> merge per (p, t)
    m2 = m.transpose(1, 3, 0, 2, 4).reshape(P, N_T, -1)   # [p, t, parts]
    s2 = s.transpose(1, 3, 0, 2, 4).reshape(P, N_T, -1)
    M = m2.max(axis=-1)                                    # [p, t]
    Ssum = (s2 * np.exp(m2 - M[..., None])).sum(axis=-1)
    lse = M + np.log(Ssum)                                 # [p, t]

    tgt_logit = tv.sum(axis=(0, 2, 4))                     # [p, t]
    loss = float((lse - tgt_logit).mean())
    return np.array(loss, dtype=np.float32)


def kernel(hidden_states, targets, weight_matrix):
    nc = _get_compiled()
    in_maps = _prep_inputs(hidden_states, targets, weight_matrix)
    res = run_bass_kernel_spmd(nc, in_maps, core_ids=list(range(NCORES)))
    return _combine(res.results)


# revision 3
# speedup vs baseline: 1.3424x; 1.3424x over previous
"""Chunked cross-entropy loss on 8 TRN2 NeuronCores (Bass/Tile).

Strategy (vocab/tensor parallel):
  - weight_matrix [V=131072, D=2048] sharded along vocab across 8 cores
    (16384 rows each); hidden_states replicated (each core computes all
    N=8192 token logits for its vocab shard).
  - Per core: tiled bf16 matmul h @ Wc^T with fp32 PSUM accumulation.
    For every [128 tok x 512 voc] logits tile the device emits
    (-max, sum_exp(logit - max)) per token row, plus the target logit
    contribution (mask-select against the live PSUM tile: exactly one
    (core, tile) contains each token's target column; all others add 0).
  - Host: float64 logsumexp-merge of the 8*8*4 = 256 per-token partials per
    token, target logit = sum of contributions, loss = mean(lse - tgt).

Numerics: bf16 inputs / fp32 accumulation. Per-tile true max means no
fixed-shift overflow/underflow risk for any input distribution.
"""

import numpy as np
import ml_dtypes

import concourse.bass as bass
import concourse.mybir as mybir
import concourse.tile as tile
from concourse import bacc
from concourse.bass_utils import run_bass_kernel_spmd

# Problem shape (hardcoded per contract).
B, S, D, V = 4, 2048, 2048, 131072
N_TOK = B * S                  # 8192 tokens
NCORES = 8
P = 128                        # partitions
VSHARD = V // NCORES           # 16384 vocab rows per core
MMF = 512                      # matmul moving free dim (one PSUM bank fp32)

# Default tiling: vocab block 2048 (resident in SBUF), 4 PSUM banks per pair.
N_K = D // P                   # 16 contraction tiles
N_J = 4                        # 512-col logits tiles per vocab block
VB = N_J * MMF                 # 2048 vocab block
N_VB = VSHARD // VB            # 8 vocab blocks per core
N_T = N_TOK // P               # 64 token tiles

BF16 = ml_dtypes.bfloat16
FP8 = ml_dtypes.float8_e4m3
USE_FP8 = True   # DoubleRow fp8 matmul (~1.5x PE); bf16 fallback if False

_COMPILED = {}


def build_nc(n_t=N_T, n_k=N_K, n_vb=N_VB, n_j=N_J, num_devices=NCORES,
             w_bufs=2, use_fp8=None):
    """Build + compile the per-core Bass program (SPMD: same program on all
    cores, per-core data differs)."""
    if use_fp8 is None:
        use_fp8 = USE_FP8
    mm_dt = mybir.dt.float8e4 if use_fp8 else mybir.dt.bfloat16
    vb = n_j * MMF
    d = n_k * P
    nc = bacc.Bacc("TRN2", target_bir_lowering=False, debug=False,
                   num_devices=num_devices)

    wt = nc.dram_tensor("wt", [P, n_vb, n_k, vb], mm_dt,
                        kind="ExternalInput")
    ht = nc.dram_tensor("ht", [n_t, P, n_k, P], mm_dt,
                        kind="ExternalInput")
    tg = nc.dram_tensor("tg", [P, n_vb, n_t], mybir.dt.float32,
                        kind="ExternalInput")
    io = nc.dram_tensor("io", [P, vb], mybir.dt.float32,
                        kind="ExternalInput")
    negm = nc.dram_tensor("negm", [P, n_vb, n_t, n_j], mybir.dt.float32,
                          kind="ExternalOutput")
    ssum = nc.dram_tensor("ssum", [P, n_vb, n_t, n_j], mybir.dt.float32,
                          kind="ExternalOutput")
    tgtv = nc.dram_tensor("tgtv", [P, n_vb, n_t, n_j], mybir.dt.float32,
                          kind="ExternalOutput")

    with tile.TileContext(nc) as tc:
        with (
            tc.tile_pool(name="wp", bufs=w_bufs) as wp,
            tc.tile_pool(name="hp", bufs=3) as hp,
            tc.tile_pool(name="pp", bufs=2, space=bass.MemorySpace.PSUM) as pp,
            tc.tile_pool(name="scr", bufs=2) as scr,
            tc.tile_pool(name="cst", bufs=1) as cst,
        ):
            iota_t = cst.tile([P, vb], mybir.dt.float32)
            nc.sync.dma_start(iota_t[:], io.ap())
            tg_t = cst.tile([P, n_vb, n_t], mybir.dt.float32)
            nc.sync.dma_start(tg_t[:], tg.ap())
            negm_t = cst.tile([P, n_vb, n_t, n_j], mybir.dt.float32)
            ssum_t = cst.tile([P, n_vb, n_t, n_j], mybir.dt.float32)
            tgtv_t = cst.tile([P, n_vb, n_t, n_j], mybir.dt.float32)

            for ivb in range(n_vb):
                # vocab-block weights stay resident for the whole token loop;
                # gpsimd (SWDGE) ring so the big load never head-of-line
                # blocks the token-tile loads on the sync (HWDGE) ring.
                w_t = wp.tile([P, n_k, vb], mm_dt)
                nc.gpsimd.dma_start(w_t[:], wt.ap()[:, ivb])
                for t in range(n_t):
                    h_t = hp.tile([P, n_k, P], mm_dt)
                    nc.sync.dma_start(h_t[:], ht.ap()[t])
                    ps = pp.tile([P, n_j, MMF], mybir.dt.float32)
                    for j in range(n_j):
                        if use_fp8:
                            # DoubleRow: 2 packed k-tiles (256 contraction)
                            # per instruction, 2 fp8 MACs/cell/cycle.
                            for kk in range(n_k // 2):
                                nc.tensor.matmul(
                                    ps[:, j, :],
                                    h_t[:, 2 * kk:2 * kk + 2, :],
                                    w_t[:, 2 * kk:2 * kk + 2,
                                        j * MMF:(j + 1) * MMF],
                                    start=(kk == 0),
                                    stop=(kk == n_k // 2 - 1),
                                    perf_mode=mybir.MatmulPerfMode.DoubleRow,
                                )
                        else:
                            for k in range(n_k):
                                nc.tensor.matmul(
                                    ps[:, j, :],
                                    h_t[:, k, :],
                                    w_t[:, k, j * MMF:(j + 1) * MMF],
                                    start=(k == 0),
                                    stop=(k == n_k - 1),
                                )
                    # per-tile -max for all n_j tiles in one reduce
                    nm = negm_t[:, ivb, t, :]
                    nc.vector.reduce_max(nm, ps[:], axis=mybir.AxisListType.X,
                                         negate=True)
                    for j in range(n_j):
                        es = scr.tile([P, MMF], mybir.dt.float32)
                        nc.scalar.activation(
                            es[:], ps[:, j, :], mybir.ActivationFunctionType.Exp,
                            bias=negm_t[:, ivb, t, j:j + 1],
                            accum_out=ssum_t[:, ivb, t, j:j + 1])
                        # target logit hit: (iota == tgt_col) * logits, summed
                        mo = scr.tile([P, MMF], mybir.dt.float32)
                        nc.vector.scalar_tensor_tensor(
                            out=mo[:], in0=iota_t[:, j * MMF:(j + 1) * MMF],
                            scalar=tg_t[:, ivb, t:t + 1], in1=ps[:, j, :],
                            op0=mybir.AluOpType.is_equal,
                            op1=mybir.AluOpType.mult,
                            accum_out=tgtv_t[:, ivb, t, j:j + 1])

            nc.sync.dma_start(negm.ap(), negm_t[:])
            nc.sync.dma_start(ssum.ap(), ssum_t[:])
            nc.sync.dma_start(tgtv.ap(), tgtv_t[:])

    nc.compile()
    return nc


def _get_compiled():
    key = "full"
    if key not in _COMPILED:
        _COMPILED[key] = build_nc()
    return _COMPILED[key]


def _prep_inputs(hidden_states, targets, weight_matrix):
    """Host-side shard + layout prep. Returns per-core in_maps."""
    h = np.ascontiguousarray(np.asarray(hidden_states, dtype=np.float32)
                             ).reshape(N_TOK, D)
    tgt = np.asarray(targets).reshape(N_TOK).astype(np.int64)
    W = np.asarray(weight_matrix, dtype=np.float32)

    mm_np = FP8 if USE_FP8 else BF16
    # h blocked: [t, p(d within k-tile), k, m(token within tile)]
    hb = np.ascontiguousarray(
        h.astype(mm_np).reshape(N_T, P, N_K, P).transpose(0, 3, 2, 1))

    iota = np.ascontiguousarray(
        np.broadcast_to(np.arange(VB, dtype=np.float32), (P, VB)))

    tl = tgt.reshape(N_T, P)  # [t, p]
    vb_off = (np.arange(N_VB, dtype=np.int64) * VB)[None, :, None]

    in_maps = []
    for c in range(NCORES):
        Wc = W[c * VSHARD:(c + 1) * VSHARD]
        wb = np.ascontiguousarray(
            Wc.astype(mm_np).reshape(N_VB, VB, N_K, P).transpose(3, 0, 2, 1))
        tgl = (tl.T[:, None, :] - c * VSHARD - vb_off).astype(np.float32)
        in_maps.append({"wt": wb, "ht": hb, "tg": np.ascontiguousarray(tgl),
                        "io": iota})
    return in_maps


def _combine(results):
    """float64 logsumexp-merge of per-core per-tile partials -> scalar loss."""
    m = np.stack([-r["negm"].astype(np.float64) for r in results])  # [C,P,vb,t,j]
    s = np.stack([r["ssum"].astype(np.float64) for r in results])
    tv = np.stack([r["tgtv"].astype(np.float64) for r in results])

    # partial axes: core, vb, j -> merge per (p, t)
    m2 = m.transpose(1, 3, 0, 2, 4).reshape(P, N_T, -1)   # [p, t, parts]
    s2 = s.transpose(1, 3, 0, 2, 4).reshape(P, N_T, -1)
    M = m2.max(axis=-1)                                    # [p, t]
    Ssum = (s2 * np.exp(m2 - M[..., None])).sum(axis=-1)
    lse = M + np.log(Ssum)                                 # [p, t]

    tgt_logit = tv.sum(axis=(0, 2, 4))                     # [p, t]
    loss = float((lse - tgt_logit).mean())
    return np.array(loss, dtype=np.float32)


def kernel(hidden_states, targets, weight_matrix):
    nc = _get_compiled()
    in_maps = _prep_inputs(hidden_states, targets, weight_matrix)
    res = run_bass_kernel_spmd(nc, in_maps, core_ids=list(range(NCORES)))
    return _combine(res.results)


# revision 5
# speedup vs baseline: 1.8880x; 1.4064x over previous
"""Chunked cross-entropy loss on 8 TRN2 NeuronCores (Bass/Tile).

Strategy (vocab/tensor parallel):
  - weight_matrix [V=131072, D=2048] sharded along vocab across 8 cores
    (16384 rows each); hidden_states replicated (each core computes all
    N=8192 token logits for its vocab shard).
  - Per core: tiled matmul h @ Wc^T (fp8-e4m3 DoubleRow, 2 packed k-tiles
    per instruction; bf16 fallback via USE_FP8=False) with fp32 PSUM
    accumulation. For every [128 tok x 512 voc] logits tile the device emits
    (-max, sum_exp(logit - max)) per token row, plus the target logit
    contribution (mask-select against the live PSUM tile: exactly one
    (core, tile) contains each token's target column; all others add 0).
  - Host: float64 logsumexp-merge of the 8*8*4 = 256 per-token partials per
    token, target logit = sum of contributions, loss = mean(lse - tgt).

Numerics: fp8/bf16 inputs, fp32 accumulation. Per-tile true max means no
fixed-shift overflow/underflow risk for any input distribution. Measured
vs fp32 reference: rel err 7.3e-4 (fp8) / 2.4e-6 (bf16); HW exec 6.10 ms
(fp8) / 8.18 ms (bf16), PE-bound (chip power-throttles to ~1.92 GHz after
~2 ms of sustained dense matmul).
"""

import numpy as np
import ml_dtypes

import concourse.bass as bass
import concourse.mybir as mybir
import concourse.tile as tile
from concourse import bacc
from concourse.bass_utils import run_bass_kernel_spmd

# Problem shape (hardcoded per contract).
B, S, D, V = 4, 2048, 2048, 131072
N_TOK = B * S                  # 8192 tokens
NCORES = 8
P = 128                        # partitions
VSHARD = V // NCORES           # 16384 vocab rows per core
MMF = 512                      # matmul moving free dim (one PSUM bank fp32)

# Default tiling: vocab block 2048 (resident in SBUF), 4 PSUM banks per pair.
N_K = D // P                   # 16 contraction tiles
N_J = 4                        # 512-col logits tiles per vocab block
VB = N_J * MMF                 # 2048 vocab block
N_VB = VSHARD // VB            # 8 vocab blocks per core
N_T = N_TOK // P               # 64 token tiles

BF16 = ml_dtypes.bfloat16
FP8 = ml_dtypes.float8_e4m3
USE_FP8 = True   # DoubleRow fp8 matmul (~1.5x PE); bf16 fallback if False

_COMPILED = {}


def build_nc(n_t=N_T, n_k=N_K, n_vb=N_VB, n_j=N_J, num_devices=NCORES,
             w_bufs=2, use_fp8=None):
    """Build + compile the per-core Bass program (SPMD: same program on all
    cores, per-core data differs)."""
    if use_fp8 is None:
        use_fp8 = USE_FP8
    mm_dt = mybir.dt.float8e4 if use_fp8 else mybir.dt.bfloat16
    vb = n_j * MMF
    d = n_k * P
    nc = bacc.Bacc("TRN2", target_bir_lowering=False, debug=False,
                   num_devices=num_devices)

    wt = nc.dram_tensor("wt", [P, n_vb, n_k, vb], mm_dt,
                        kind="ExternalInput")
    ht = nc.dram_tensor("ht", [n_t, P, n_k, P], mm_dt,
                        kind="ExternalInput")
    tg = nc.dram_tensor("tg", [P, n_vb, n_t], mybir.dt.float32,
                        kind="ExternalInput")
    io = nc.dram_tensor("io", [P, vb], mybir.dt.float32,
                        kind="ExternalInput")
    negm = nc.dram_tensor("negm", [P, n_vb, n_t, n_j], mybir.dt.float32,
                          kind="ExternalOutput")
    ssum = nc.dram_tensor("ssum", [P, n_vb, n_t, n_j], mybir.dt.float32,
                          kind="ExternalOutput")
    tgtv = nc.dram_tensor("tgtv", [P, n_vb, n_t, n_j], mybir.dt.float32,
                          kind="ExternalOutput")

    with tile.TileContext(nc) as tc:
        with (
            tc.tile_pool(name="wp", bufs=w_bufs) as wp,
            tc.tile_pool(name="hp", bufs=3) as hp,
            tc.tile_pool(name="pp", bufs=8, space=bass.MemorySpace.PSUM) as pp,
            tc.tile_pool(name="scr", bufs=2) as scr,
            tc.tile_pool(name="cst", bufs=1) as cst,
        ):
            iota_t = cst.tile([P, vb], mybir.dt.float32)
            nc.sync.dma_start(iota_t[:], io.ap())
            tg_t = cst.tile([P, n_vb, n_t], mybir.dt.float32)
            nc.sync.dma_start(tg_t[:], tg.ap())
            negm_t = cst.tile([P, n_vb, n_t, n_j], mybir.dt.float32)
            ssum_t = cst.tile([P, n_vb, n_t, n_j], mybir.dt.float32)
            tgtv_t = cst.tile([P, n_vb, n_t, n_j], mybir.dt.float32)

            for ivb in range(n_vb):
                # vocab-block weights stay resident for the whole token loop;
                # gpsimd (SWDGE) ring so the big load never head-of-line
                # blocks the token-tile loads on the sync (HWDGE) ring.
                w_t = wp.tile([P, n_k, vb], mm_dt)
                nc.gpsimd.dma_start(w_t[:], wt.ap()[:, ivb])
                for t in range(n_t):
                    h_t = hp.tile([P, n_k, P], mm_dt)
                    nc.sync.dma_start(h_t[:], ht.ap()[t])
                    for j in range(n_j):
                        # per-bank PSUM tile: each bank frees as soon as its
                        # own reduce/exp/select finish, keeping PE fed.
                        ps = pp.tile([P, MMF], mybir.dt.float32)
                        if use_fp8:
                            # DoubleRow: 2 packed k-tiles (256 contraction)
                            # per instruction, 2 fp8 MACs/cell/cycle.
                            for kk in range(n_k // 2):
                                nc.tensor.matmul(
                                    ps[:],
                                    h_t[:, 2 * kk:2 * kk + 2, :],
                                    w_t[:, 2 * kk:2 * kk + 2,
                                        j * MMF:(j + 1) * MMF],
                                    start=(kk == 0),
                                    stop=(kk == n_k // 2 - 1),
                                    perf_mode=mybir.MatmulPerfMode.DoubleRow,
                                )
                        else:
                            for k in range(n_k):
                                nc.tensor.matmul(
                                    ps[:],
                                    h_t[:, k, :],
                                    w_t[:, k, j * MMF:(j + 1) * MMF],
                                    start=(k == 0),
                                    stop=(k == n_k - 1),
                                )
                        nc.vector.reduce_max(negm_t[:, ivb, t, j:j + 1],
                                             ps[:], axis=mybir.AxisListType.X,
                                             negate=True)
                        es = scr.tile([P, MMF], mybir.dt.float32)
                        nc.scalar.activation(
                            es[:], ps[:], mybir.ActivationFunctionType.Exp,
                            bias=negm_t[:, ivb, t, j:j + 1],
                            accum_out=ssum_t[:, ivb, t, j:j + 1])
                        # target logit hit: (iota == tgt_col) * logits, summed
                        mo = scr.tile([P, MMF], mybir.dt.float32)
                        nc.vector.scalar_tensor_tensor(
                            out=mo[:], in0=iota_t[:, j * MMF:(j + 1) * MMF],
                            scalar=tg_t[:, ivb, t:t + 1], in1=ps[:],
                            op0=mybir.AluOpType.is_equal,
                            op1=mybir.AluOpType.mult,
                            accum_out=tgtv_t[:, ivb, t, j:j + 1])

            nc.sync.dma_start(negm.ap(), negm_t[:])
            nc.sync.dma_start(ssum.ap(), ssum_t[:])
            nc.sync.dma_start(tgtv.ap(), tgtv_t[:])

    nc.compile()
    return nc


def _get_compiled():
    key = "full"
    if key not in _COMPILED:
        _COMPILED[key] = build_nc()
    return _COMPILED[key]


def _prep_inputs(hidden_states, targets, weight_matrix):
    """Host-side shard + layout prep. Returns per-core in_maps."""
    h = np.ascontiguousarray(np.asarray(hidden_states, dtype=np.float32)
                             ).reshape(N_TOK, D)
    tgt = np.asarray(targets).reshape(N_TOK).astype(np.int64)
    W = np.asarray(weight_matrix, dtype=np.float32)

    mm_np = FP8 if USE_FP8 else BF16
    # h blocked: [t, p(d within k-tile), k, m(token within tile)]
    hb = np.ascontiguousarray(
        h.astype(mm_np).reshape(N_T, P, N_K, P).transpose(0, 3, 2, 1))

    iota = np.ascontiguousarray(
        np.broadcast_to(np.arange(VB, dtype=np.float32), (P, VB)))

    tl = tgt.reshape(N_T, P)  # [t, p]
    vb_off = (np.arange(N_VB, dtype=np.int64) * VB)[None, :, None]

    in_maps = []
    for c in range(NCORES):
        Wc = W[c * VSHARD:(c + 1) * VSHARD]
        wb = np.ascontiguousarray(
            Wc.astype(mm_np).reshape(N_VB, VB, N_K, P).transpose(3, 0, 2, 1))
        tgl = (tl.T[:, None, :] - c * VSHARD - vb_off).astype(np.float32)
        in_maps.append({"wt": wb, "ht": hb, "tg": np.ascontiguousarray(tgl),
                        "io": iota})
    return in_maps


def _combine(results):
    """float64 logsumexp-merge of per-core per-tile partials -> scalar loss."""
    m = np.stack([-r["negm"].astype(np.float64) for r in results])  # [C,P,vb,t,j]
    s = np.stack([r["ssum"].astype(np.float64) for r in results])
    tv = np.stack([r["tgtv"].astype(np.float64) for r in results])

    # partial axes: core, vb, j -> merge per (p, t)
    m2 = m.transpose(1, 3, 0, 2, 4).reshape(P, N_T, -1)   # [p, t, parts]
    s2 = s.transpose(1, 3, 0, 2, 4).reshape(P, N_T, -1)
    M = m2.max(axis=-1)                                    # [p, t]
    Ssum = (s2 * np.exp(m2 - M[..., None])).sum(axis=-1)
    lse = M + np.log(Ssum)                                 # [p, t]

    tgt_logit = tv.sum(axis=(0, 2, 4))                     # [p, t]
    loss = float((lse - tgt_logit).mean())
    return np.array(loss, dtype=np.float32)


def kernel(hidden_states, targets, weight_matrix):
    nc = _get_compiled()
    in_maps = _prep_inputs(hidden_states, targets, weight_matrix)
    res = run_bass_kernel_spmd(nc, in_maps, core_ids=list(range(NCORES)))
    return _combine(res.results)


# revision 6
# speedup vs baseline: 2.2791x; 1.2071x over previous
"""Chunked cross-entropy loss on 8 TRN2 NeuronCores (Bass/Tile).

Strategy (vocab/tensor parallel):
  - weight_matrix [V=131072, D=2048] sharded along vocab across 8 cores
    (16384 rows each); hidden_states replicated (each core computes all
    N=8192 token logits for its vocab shard).
  - Per core: tiled matmul h @ Wc^T (fp8-e4m3 DoubleRow, 2 packed k-tiles
    per instruction; bf16 fallback via USE_FP8=False) with fp32 PSUM
    accumulation. For every [128 tok x 512 voc] logits tile the device emits
    (-max, sum_exp(logit - max)) per token row, plus the target logit
    contribution (mask-select against the live PSUM tile: exactly one
    (core, tile) contains each token's target column; all others add 0).
  - Host: float64 logsumexp-merge of the 8*8*4 = 256 per-token partials per
    token, target logit = sum of contributions, loss = mean(lse - tgt).

Numerics: fp8/bf16 inputs, fp32 accumulation. Per-tile true max means no
fixed-shift overflow/underflow risk for any input distribution. Measured
vs fp32 reference: rel err 7.3e-4 (fp8) / 2.4e-6 (bf16); HW exec 4.33 ms
(fp8, 98% PE-busy at the DoubleRow roofline) / 8.18 ms (bf16). Per-bank
PSUM tiles (bufs=8) are load-bearing: one grouped [128,4,512] PSUM tile
released only after all consumers finished, starving the PE (71% busy,
HAM re-throttle oscillation) and costing 1.4x.
"""

import numpy as np
import ml_dtypes

import concourse.bass as bass
import concourse.mybir as mybir
import concourse.tile as tile
from concourse import bacc
from concourse.bass_utils import run_bass_kernel_spmd

# Problem shape (hardcoded per contract).
B, S, D, V = 4, 2048, 2048, 131072
N_TOK = B * S                  # 8192 tokens
NCORES = 8
P = 128                        # partitions
VSHARD = V // NCORES           # 16384 vocab rows per core
MMF = 512                      # matmul moving free dim (one PSUM bank fp32)

# Default tiling: vocab block 2048 (resident in SBUF), per-bank PSUM tiles.
N_K = D // P                   # 16 contraction tiles
N_J = 4                        # 512-col logits tiles per vocab block
VB = N_J * MMF                 # 2048 vocab block
N_VB = VSHARD // VB            # 8 vocab blocks per core
N_T = N_TOK // P               # 64 token tiles

BF16 = ml_dtypes.bfloat16
FP8 = ml_dtypes.float8_e4m3
USE_FP8 = True   # DoubleRow fp8 matmul (~1.5x PE); bf16 fallback if False

_COMPILED = {}


def build_nc(n_t=N_T, n_k=N_K, n_vb=N_VB, n_j=N_J, num_devices=NCORES,
             w_bufs=2, use_fp8=None):
    """Build + compile the per-core Bass program (SPMD: same program on all
    cores, per-core data differs)."""
    if use_fp8 is None:
        use_fp8 = USE_FP8
    mm_dt = mybir.dt.float8e4 if use_fp8 else mybir.dt.bfloat16
    vb = n_j * MMF
    d = n_k * P
    nc = bacc.Bacc("TRN2", target_bir_lowering=False, debug=False,
                   num_devices=num_devices)

    wt = nc.dram_tensor("wt", [P, n_vb, n_k, vb], mm_dt,
                        kind="ExternalInput")
    ht = nc.dram_tensor("ht", [n_t, P, n_k, P], mm_dt,
                        kind="ExternalInput")
    tg = nc.dram_tensor("tg", [P, n_vb, n_t], mybir.dt.float32,
                        kind="ExternalInput")
    io = nc.dram_tensor("io", [P, vb], mybir.dt.float32,
                        kind="ExternalInput")
    negm = nc.dram_tensor("negm", [P, n_vb, n_t, n_j], mybir.dt.float32,
                          kind="ExternalOutput")
    ssum = nc.dram_tensor("ssum", [P, n_vb, n_t, n_j], mybir.dt.float32,
                          kind="ExternalOutput")
    tgtv = nc.dram_tensor("tgtv", [P, n_vb, n_t, n_j], mybir.dt.float32,
                          kind="ExternalOutput")

    with tile.TileContext(nc) as tc:
        with (
            tc.tile_pool(name="wp", bufs=w_bufs) as wp,
            tc.tile_pool(name="hp", bufs=3) as hp,
            tc.tile_pool(name="pp", bufs=8, space=bass.MemorySpace.PSUM) as pp,
            tc.tile_pool(name="scr", bufs=2) as scr,
            tc.tile_pool(name="cst", bufs=1) as cst,
        ):
            iota_t = cst.tile([P, vb], mybir.dt.float32)
            nc.sync.dma_start(iota_t[:], io.ap())
            tg_t = cst.tile([P, n_vb, n_t], mybir.dt.float32)
            nc.sync.dma_start(tg_t[:], tg.ap())
            negm_t = cst.tile([P, n_vb, n_t, n_j], mybir.dt.float32)
            ssum_t = cst.tile([P, n_vb, n_t, n_j], mybir.dt.float32)
            tgtv_t = cst.tile([P, n_vb, n_t, n_j], mybir.dt.float32)

            for ivb in range(n_vb):
                # vocab-block weights stay resident for the whole token loop;
                # gpsimd (SWDGE) ring so the big load never head-of-line
                # blocks the token-tile loads on the sync (HWDGE) ring.
                w_t = wp.tile([P, n_k, vb], mm_dt)
                nc.gpsimd.dma_start(w_t[:], wt.ap()[:, ivb])
                for t in range(n_t):
                    h_t = hp.tile([P, n_k, P], mm_dt)
                    nc.sync.dma_start(h_t[:], ht.ap()[t])
                    for j in range(n_j):
                        # per-bank PSUM tile: each bank frees as soon as its
                        # own reduce/exp/select finish, keeping PE fed.
                        ps = pp.tile([P, MMF], mybir.dt.float32)
                        if use_fp8:
                            # DoubleRow: 2 packed k-tiles (256 contraction)
                            # per instruction, 2 fp8 MACs/cell/cycle.
                            for kk in range(n_k // 2):
                                nc.tensor.matmul(
                                    ps[:],
                                    h_t[:, 2 * kk:2 * kk + 2, :],
                                    w_t[:, 2 * kk:2 * kk + 2,
                                        j * MMF:(j + 1) * MMF],
                                    start=(kk == 0),
                                    stop=(kk == n_k // 2 - 1),
                                    perf_mode=mybir.MatmulPerfMode.DoubleRow,
                                )
                        else:
                            for k in range(n_k):
                                nc.tensor.matmul(
                                    ps[:],
                                    h_t[:, k, :],
                                    w_t[:, k, j * MMF:(j + 1) * MMF],
                                    start=(k == 0),
                                    stop=(k == n_k - 1),
                                )
                        nc.vector.reduce_max(negm_t[:, ivb, t, j:j + 1],
                                             ps[:], axis=mybir.AxisListType.X,
                                             negate=True)
                        es = scr.tile([P, MMF], mybir.dt.float32)
                        nc.scalar.activation(
                            es[:], ps[:], mybir.ActivationFunctionType.Exp,
                            bias=negm_t[:, ivb, t, j:j + 1],
                            accum_out=ssum_t[:, ivb, t, j:j + 1])
                        # target logit hit: (iota == tgt_col) * logits, summed
                        mo = scr.tile([P, MMF], mybir.dt.float32)
                        nc.vector.scalar_tensor_tensor(
                            out=mo[:], in0=iota_t[:, j * MMF:(j + 1) * MMF],
                            scalar=tg_t[:, ivb, t:t + 1], in1=ps[:],
                            op0=mybir.AluOpType.is_equal,
                            op1=mybir.AluOpType.mult,
                            accum_out=tgtv_t[:, ivb, t, j:j + 1])

            nc.sync.dma_start(negm.ap(), negm_t[:])
            nc.sync.dma_start(ssum.ap(), ssum_t[:])
            nc.sync.dma_start(tgtv.ap(), tgtv_t[:])

    nc.compile()
    return nc


def _get_compiled():
    key = "full"
    if key not in _COMPILED:
        _COMPILED[key] = build_nc()
    return _COMPILED[key]


def _prep_inputs(hidden_states, targets, weight_matrix):
    """Host-side shard + layout prep. Returns per-core in_maps."""
    h = np.ascontiguousarray(np.asarray(hidden_states, dtype=np.float32)
                             ).reshape(N_TOK, D)
    tgt = np.asarray(targets).reshape(N_TOK).astype(np.int64)
    W = np.asarray(weight_matrix, dtype=np.float32)

    mm_np = FP8 if USE_FP8 else BF16
    # h blocked: [t, p(d within k-tile), k, m(token within tile)]
    hb = np.ascontiguousarray(
        h.astype(mm_np).reshape(N_T, P, N_K, P).transpose(0, 3, 2, 1))

    iota = np.ascontiguousarray(
        np.broadcast_to(np.arange(VB, dtype=np.float32), (P, VB)))

    tl = tgt.reshape(N_T, P)  # [t, p]
    vb_off = (np.arange(N_VB, dtype=np.int64) * VB)[None, :, None]

    in_maps = []
    for c in range(NCORES):
        Wc = W[c * VSHARD:(c + 1) * VSHARD]
        wb = np.ascontiguousarray(
            Wc.astype(mm_np).reshape(N_VB, VB, N_K, P).transpose(3, 0, 2, 1))
        tgl = (tl.T[:, None, :] - c * VSHARD - vb_off).astype(np.float32)
        in_maps.append({"wt": wb, "ht": hb, "tg": np.ascontiguousarray(tgl),
                        "io": iota})
    return in_maps


def _combine(results):
    """float64 logsumexp-merge of per-core per-tile partials -> scalar loss."""
    m = np.stack([-r["negm"].astype(np.float64) for r in results])  # [C,P,vb,t,j]
    s = np.stack([r["ssum"].astype(np.float64) for r in results])
    tv = np.stack([r["tgtv"].astype(np.float64) for r in results])

    # partial axes: core, vb, j -> merge per (p, t)
    m2 = m.transpose(1, 3, 0, 2, 4).reshape(P, N_T, -1)   # [p, t, parts]
    s2 = s.transpose(1, 3, 0, 2, 4).reshape(P, N_T, -1)
    M = m2.max(axis=-1)                                    # [p, t]
    Ssum = (s2 * np.exp(m2 - M[..., None])).sum(axis=-1)
    lse = M + np.log(Ssum)                                 # [p, t]

    tgt_logit = tv.sum(axis=(0, 2, 4))                     # [p, t]
    loss = float((lse - tgt_logit).mean())
    return np.array(loss, dtype=np.float32)


def kernel(hidden_states, targets, weight_matrix):
    nc = _get_compiled()
    in_maps = _prep_inputs(hidden_states, targets, weight_matrix)
    res = run_bass_kernel_spmd(nc, in_maps, core_ids=list(range(NCORES)))
    return _combine(res.results)


# revision 7
# speedup vs baseline: 2.2887x; 1.0042x over previous
"""Chunked cross-entropy loss on 8 TRN2 NeuronCores (Bass/Tile).

Strategy (vocab/tensor parallel):
  - weight_matrix [V=131072, D=2048] sharded along vocab across 8 cores
    (16384 rows each); hidden_states replicated (each core computes all
    N=8192 token logits for its vocab shard).
  - Per core: tiled matmul h @ Wc^T (fp8-e4m3 DoubleRow, 2 packed k-tiles
    per instruction; bf16 fallback via USE_FP8=False) with fp32 PSUM
    accumulation. For every [128 tok x 512 voc] logits tile the device emits
    (-max, sum_exp(logit - max)) per token row, plus the target logit
    contribution (mask-select against the live PSUM tile: exactly one
    (core, tile) contains each token's target column; all others add 0).
  - Host: float64 logsumexp-merge of the 8*8*4 = 256 per-token partials per
    token, target logit = sum of contributions, loss = mean(lse - tgt).

Numerics: fp8/bf16 inputs, fp32 accumulation. Per-tile true max means no
fixed-shift overflow/underflow risk for any input distribution. Measured
vs fp32 reference: rel err 7.3e-4 (fp8) / 2.4e-6 (bf16); HW exec 4.33 ms
(fp8, 98% PE-busy at the DoubleRow roofline) / 8.18 ms (bf16). Per-bank
PSUM tiles (bufs=8) are load-bearing: one grouped [128,4,512] PSUM tile
released only after all consumers finished, starving the PE (71% busy,
HAM re-throttle oscillation) and costing 1.4x.
"""

import numpy as np
import ml_dtypes

import concourse.bass as bass
import concourse.mybir as mybir
import concourse.tile as tile
from concourse import bacc
from concourse.bass_utils import run_bass_kernel_spmd

# Problem shape (hardcoded per contract).
B, S, D, V = 4, 2048, 2048, 131072
N_TOK = B * S                  # 8192 tokens
NCORES = 8
P = 128                        # partitions
VSHARD = V // NCORES           # 16384 vocab rows per core
MMF = 512                      # matmul moving free dim (one PSUM bank fp32)

# Default tiling: vocab block 2048 (resident in SBUF), per-bank PSUM tiles.
N_K = D // P                   # 16 contraction tiles
N_J = 4                        # 512-col logits tiles per vocab block
VB = N_J * MMF                 # 2048 vocab block
N_VB = VSHARD // VB            # 8 vocab blocks per core
N_T = N_TOK // P               # 64 token tiles

BF16 = ml_dtypes.bfloat16
FP8 = ml_dtypes.float8_e4m3
USE_FP8 = True   # DoubleRow fp8 matmul (~1.5x PE); bf16 fallback if False

_COMPILED = {}


def build_nc(n_t=N_T, n_k=N_K, n_vb=N_VB, n_j=N_J, num_devices=NCORES,
             w_bufs=2, use_fp8=None):
    """Build + compile the per-core Bass program (SPMD: same program on all
    cores, per-core data differs)."""
    if use_fp8 is None:
        use_fp8 = USE_FP8
    mm_dt = mybir.dt.float8e4 if use_fp8 else mybir.dt.bfloat16
    vb = n_j * MMF
    d = n_k * P
    nc = bacc.Bacc("TRN2", target_bir_lowering=False, debug=False,
                   num_devices=num_devices)

    wt = nc.dram_tensor("wt", [P, n_vb, n_k, vb], mm_dt,
                        kind="ExternalInput")
    ht = nc.dram_tensor("ht", [n_t, P, n_k, P], mm_dt,
                        kind="ExternalInput")
    tg = nc.dram_tensor("tg", [P, n_vb, n_t], mybir.dt.float32,
                        kind="ExternalInput")
    io = nc.dram_tensor("io", [P, vb], mybir.dt.float32,
                        kind="ExternalInput")
    negm = nc.dram_tensor("negm", [P, n_vb, n_t, n_j], mybir.dt.float32,
                          kind="ExternalOutput")
    ssum = nc.dram_tensor("ssum", [P, n_vb, n_t, n_j], mybir.dt.float32,
                          kind="ExternalOutput")
    tgtv = nc.dram_tensor("tgtv", [P, n_vb, n_t, n_j], mybir.dt.float32,
                          kind="ExternalOutput")

    with tile.TileContext(nc) as tc:
        with (
            tc.tile_pool(name="wp", bufs=w_bufs) as wp,
            tc.tile_pool(name="hp", bufs=3) as hp,
            tc.tile_pool(name="pp", bufs=8, space=bass.MemorySpace.PSUM) as pp,
            tc.tile_pool(name="scr", bufs=2) as scr,
            tc.tile_pool(name="cst", bufs=1) as cst,
        ):
            iota_t = cst.tile([P, vb], mybir.dt.float32)
            nc.sync.dma_start(iota_t[:], io.ap())
            tg_t = cst.tile([P, n_vb, n_t], mybir.dt.float32)
            nc.sync.dma_start(tg_t[:], tg.ap())
            negm_t = cst.tile([P, n_vb, n_t, n_j], mybir.dt.float32)
            ssum_t = cst.tile([P, n_vb, n_t, n_j], mybir.dt.float32)
            tgtv_t = cst.tile([P, n_vb, n_t, n_j], mybir.dt.float32)

            for ivb in range(n_vb):
                # vocab-block weights stay resident for the whole token loop;
                # gpsimd (SWDGE) ring so the big load never head-of-line
                # blocks the token-tile loads on the sync (HWDGE) ring.
                w_t = wp.tile([P, n_k, vb], mm_dt)
                # per-k-tile loads: first matmuls start after the first
                # k-pair lands instead of waiting for the whole 4MB block
                for k in range(n_k):
                    nc.gpsimd.dma_start(w_t[:, k, :], wt.ap()[:, ivb, k])
                for t in range(n_t):
                    h_t = hp.tile([P, n_k, P], mm_dt)
                    nc.sync.dma_start(h_t[:], ht.ap()[t])
                    for j in range(n_j):
                        # per-bank PSUM tile: each bank frees as soon as its
                        # own reduce/exp/select finish, keeping PE fed.
                        ps = pp.tile([P, MMF], mybir.dt.float32)
                        if use_fp8:
                            # DoubleRow: 2 packed k-tiles (256 contraction)
                            # per instruction, 2 fp8 MACs/cell/cycle.
                            for kk in range(n_k // 2):
                                nc.tensor.matmul(
                                    ps[:],
                                    h_t[:, 2 * kk:2 * kk + 2, :],
                                    w_t[:, 2 * kk:2 * kk + 2,
                                        j * MMF:(j + 1) * MMF],
                                    start=(kk == 0),
                                    stop=(kk == n_k // 2 - 1),
                                    perf_mode=mybir.MatmulPerfMode.DoubleRow,
                                )
                        else:
                            for k in range(n_k):
                                nc.tensor.matmul(
                                    ps[:],
                                    h_t[:, k, :],
                                    w_t[:, k, j * MMF:(j + 1) * MMF],
                                    start=(k == 0),
                                    stop=(k == n_k - 1),
                                )
                        nc.vector.reduce_max(negm_t[:, ivb, t, j:j + 1],
                                             ps[:], axis=mybir.AxisListType.X,
                                             negate=True)
                        es = scr.tile([P, MMF], mybir.dt.float32)
                        nc.scalar.activation(
                            es[:], ps[:], mybir.ActivationFunctionType.Exp,
                            bias=negm_t[:, ivb, t, j:j + 1],
                            accum_out=ssum_t[:, ivb, t, j:j + 1])
                        # target logit hit: (iota == tgt_col) * logits, summed
                        mo = scr.tile([P, MMF], mybir.dt.float32)
                        nc.vector.scalar_tensor_tensor(
                            out=mo[:], in0=iota_t[:, j * MMF:(j + 1) * MMF],
                            scalar=tg_t[:, ivb, t:t + 1], in1=ps[:],
                            op0=mybir.AluOpType.is_equal,
                            op1=mybir.AluOpType.mult,
                            accum_out=tgtv_t[:, ivb, t, j:j + 1])

            nc.sync.dma_start(negm.ap(), negm_t[:])
            nc.sync.dma_start(ssum.ap(), ssum_t[:])
            nc.sync.dma_start(tgtv.ap(), tgtv_t[:])

    nc.compile()
    return nc


def _get_compiled():
    key = "full"
    if key not in _COMPILED:
        _COMPILED[key] = build_nc()
    return _COMPILED[key]


def _prep_inputs(hidden_states, targets, weight_matrix):
    """Host-side shard + layout prep. Returns per-core in_maps."""
    h = np.ascontiguousarray(np.asarray(hidden_states, dtype=np.float32)
                             ).reshape(N_TOK, D)
    tgt = np.asarray(targets).reshape(N_TOK).astype(np.int64)
    W = np.asarray(weight_matrix, dtype=np.float32)

    mm_np = FP8 if USE_FP8 else BF16
    # h blocked: [t, p(d within k-tile), k, m(token within tile)]
    hb = np.ascontiguousarray(
        h.astype(mm_np).reshape(N_T, P, N_K, P).transpose(0, 3, 2, 1))

    iota = np.ascontiguousarray(
        np.broadcast_to(np.arange(VB, dtype=np.float32), (P, VB)))

    tl = tgt.reshape(N_T, P)  # [t, p]
    vb_off = (np.arange(N_VB, dtype=np.int64) * VB)[None, :, None]

    in_maps = []
    for c in range(NCORES):
        Wc = W[c * VSHARD:(c + 1) * VSHARD]
        wb = np.ascontiguousarray(
            Wc.astype(mm_np).reshape(N_VB, VB, N_K, P).transpose(3, 0, 2, 1))
        tgl = (tl.T[:, None, :] - c * VSHARD - vb_off).astype(np.float32)
        in_maps.append({"wt": wb, "ht": hb, "tg": np.ascontiguousarray(tgl),
                        "io": iota})
    return in_maps


def _combine(results):
    """float64 logsumexp-merge of per-core per-tile partials -> scalar loss."""
    m = np.stack([-r["negm"].astype(np.float64) for r in results])  # [C,P,vb,t,j]
    s = np.stack([r["ssum"].astype(np.float64) for r in results])
    tv = np.stack([r["tgtv"].astype(np.float64) for r in results])

    # partial axes: core, vb, j -> merge per (p, t)
    m2 = m.transpose(1, 3, 0, 2, 4).reshape(P, N_T, -1)   # [p, t, parts]
    s2 = s.transpose(1, 3, 0, 2, 4).reshape(P, N_T, -1)
    M = m2.max(axis=-1)                                    # [p, t]
    Ssum = (s2 * np.exp(m2 - M[..., None])).sum(axis=-1)
    lse = M + np.log(Ssum)                                 # [p, t]

    tgt_logit = tv.sum(axis=(0, 2, 4))                     # [p, t]
    loss = float((lse - tgt_logit).mean())
    return np.array(loss, dtype=np.float32)


def kernel(hidden_states, targets, weight_matrix):
    nc = _get_compiled()
    in_maps = _prep_inputs(hidden_states, targets, weight_matrix)
    res = run_bass_kernel_spmd(nc, in_maps, core_ids=list(range(NCORES)))
    return _combine(res.results)


# revision 9
# speedup vs baseline: 2.2919x; 1.0014x over previous
"""Chunked cross-entropy loss on 8 TRN2 NeuronCores (Bass/Tile).

Strategy (vocab/tensor parallel):
  - weight_matrix [V=131072, D=2048] sharded along vocab across 8 cores
    (16384 rows each); hidden_states replicated (each core computes all
    N=8192 token logits for its vocab shard).
  - Per core: tiled matmul h @ Wc^T (fp8-e4m3 DoubleRow, 2 packed k-tiles
    per instruction; bf16 fallback via USE_FP8=False) with fp32 PSUM
    accumulation. For every [128 tok x 512 voc] logits tile the device emits
    (-max, sum_exp(logit - max)) per token row, plus the target logit
    contribution (mask-select against the live PSUM tile: exactly one
    (core, tile) contains each token's target column; all others add 0).
  - Host: float64 logsumexp-merge of the 8*8*4 = 256 per-token partials per
    token, target logit = sum of contributions, loss = mean(lse - tgt).

Numerics: fp8/bf16 inputs, fp32 accumulation. Per-tile true max means no
fixed-shift overflow/underflow risk for any input distribution. Measured
vs fp32 reference: rel err 7.3e-4 (fp8) / 2.4e-6 (bf16); HW exec 3.58 ms
best / 4.33 ms power-throttled (fp8, 98% PE-busy, ~102% of the absolute
DoubleRow streaming floor of 3.50 ms) vs 8.18 ms (bf16). Per-bank PSUM
tiles (bufs=8) are load-bearing: one grouped [128,4,512] PSUM tile
released only after all consumers finished, starving the PE (71% busy,
HAM re-throttle oscillation) and costing 1.4x.
"""

import numpy as np
import ml_dtypes

import concourse.bass as bass
import concourse.mybir as mybir
import concourse.tile as tile
from concourse import bacc
from concourse.bass_utils import run_bass_kernel_spmd

# Problem shape (hardcoded per contract).
B, S, D, V = 4, 2048, 2048, 131072
N_TOK = B * S                  # 8192 tokens
NCORES = 8
P = 128                        # partitions
VSHARD = V // NCORES           # 16384 vocab rows per core
MMF = 512                      # matmul moving free dim (one PSUM bank fp32)

# Default tiling: vocab block 2048 (resident in SBUF), per-bank PSUM tiles.
N_K = D // P                   # 16 contraction tiles
N_J = 4                        # 512-col logits tiles per vocab block
VB = N_J * MMF                 # 2048 vocab block
N_VB = VSHARD // VB            # 8 vocab blocks per core
N_T = N_TOK // P               # 64 token tiles

BF16 = ml_dtypes.bfloat16
FP8 = ml_dtypes.float8_e4m3
USE_FP8 = True   # DoubleRow fp8 matmul (~1.5x PE); bf16 fallback if False

_COMPILED = {}


def build_nc(n_t=N_T, n_k=N_K, n_vb=N_VB, n_j=N_J, num_devices=NCORES,
             w_bufs=2, use_fp8=None):
    """Build + compile the per-core Bass program (SPMD: same program on all
    cores, per-core data differs)."""
    if use_fp8 is None:
        use_fp8 = USE_FP8
    mm_dt = mybir.dt.float8e4 if use_fp8 else mybir.dt.bfloat16
    vb = n_j * MMF
    d = n_k * P
    nc = bacc.Bacc("TRN2", target_bir_lowering=False, debug=False,
                   num_devices=num_devices)

    wt = nc.dram_tensor("wt", [P, n_vb, n_k, vb], mm_dt,
                        kind="ExternalInput")
    ht = nc.dram_tensor("ht", [n_t, P, n_k, P], mm_dt,
                        kind="ExternalInput")
    tg = nc.dram_tensor("tg", [P, n_vb, n_t], mybir.dt.float32,
                        kind="ExternalInput")
    io = nc.dram_tensor("io", [P, vb], mybir.dt.float32,
                        kind="ExternalInput")
    negm = nc.dram_tensor("negm", [P, n_vb, n_t, n_j], mybir.dt.float32,
                          kind="ExternalOutput")
    ssum = nc.dram_tensor("ssum", [P, n_vb, n_t, n_j], mybir.dt.float32,
                          kind="ExternalOutput")
    tgtv = nc.dram_tensor("tgtv", [P, n_vb, n_t, n_j], mybir.dt.float32,
                          kind="ExternalOutput")

    with tile.TileContext(nc) as tc:
        with (
            tc.tile_pool(name="wp", bufs=w_bufs) as wp,
            tc.tile_pool(name="hp", bufs=3) as hp,
            tc.tile_pool(name="pp", bufs=8, space=bass.MemorySpace.PSUM) as pp,
            tc.tile_pool(name="scr", bufs=2) as scr,
            tc.tile_pool(name="cst", bufs=1) as cst,
        ):
            iota_t = cst.tile([P, vb], mybir.dt.float32)
            tg_t = cst.tile([P, n_vb, n_t], mybir.dt.float32)
            negm_t = cst.tile([P, n_vb, n_t, n_j], mybir.dt.float32)
            ssum_t = cst.tile([P, n_vb, n_t, n_j], mybir.dt.float32)
            tgtv_t = cst.tile([P, n_vb, n_t, n_j], mybir.dt.float32)

            for ivb in range(n_vb):
                # vocab-block weights stay resident for the whole token loop;
                # gpsimd (SWDGE) ring so the big load never head-of-line
                # blocks the token-tile loads on the sync (HWDGE) ring.
                w_t = wp.tile([P, n_k, vb], mm_dt)
                # per-k-tile loads: first matmuls start after the first
                # k-pair lands instead of waiting for the whole 4MB block.
                # First block alternates gpsimd/scalar rings to halve the
                # serial descriptor-gen latency the PE races at startup.
                for k in range(n_k):
                    eng = nc.scalar if (ivb == 0 and k % 2 == 1) else nc.gpsimd
                    eng.dma_start(w_t[:, k, :], wt.ap()[:, ivb, k])
                if ivb == 0:
                    # behind the critical W loads on the scalar ring; needed
                    # only once the first PSUM bank completes
                    nc.scalar.dma_start(iota_t[:], io.ap())
                    nc.scalar.dma_start(tg_t[:], tg.ap())
                for t in range(n_t):
                    h_t = hp.tile([P, n_k, P], mm_dt)
                    nc.sync.dma_start(h_t[:], ht.ap()[t])
                    for j in range(n_j):
                        # per-bank PSUM tile: each bank frees as soon as its
                        # own reduce/exp/select finish, keeping PE fed.
                        ps = pp.tile([P, MMF], mybir.dt.float32)
                        if use_fp8:
                            # DoubleRow: 2 packed k-tiles (256 contraction)
                            # per instruction, 2 fp8 MACs/cell/cycle.
                            for kk in range(n_k // 2):
                                nc.tensor.matmul(
                                    ps[:],
                                    h_t[:, 2 * kk:2 * kk + 2, :],
                                    w_t[:, 2 * kk:2 * kk + 2,
                                        j * MMF:(j + 1) * MMF],
                                    start=(kk == 0),
                                    stop=(kk == n_k // 2 - 1),
                                    perf_mode=mybir.MatmulPerfMode.DoubleRow,
                                )
                        else:
                            for k in range(n_k):
                                nc.tensor.matmul(
                                    ps[:],
                                    h_t[:, k, :],
                                    w_t[:, k, j * MMF:(j + 1) * MMF],
                                    start=(k == 0),
                                    stop=(k == n_k - 1),
                                )
                        nc.vector.reduce_max(negm_t[:, ivb, t, j:j + 1],
                                             ps[:], axis=mybir.AxisListType.X,
                                             negate=True)
                        es = scr.tile([P, MMF], mybir.dt.float32)
                        nc.scalar.activation(
                            es[:], ps[:], mybir.ActivationFunctionType.Exp,
                            bias=negm_t[:, ivb, t, j:j + 1],
                            accum_out=ssum_t[:, ivb, t, j:j + 1])
                        # target logit hit: (iota == tgt_col) * logits, summed
                        mo = scr.tile([P, MMF], mybir.dt.float32)
                        nc.vector.scalar_tensor_tensor(
                            out=mo[:], in0=iota_t[:, j * MMF:(j + 1) * MMF],
                            scalar=tg_t[:, ivb, t:t + 1], in1=ps[:],
                            op0=mybir.AluOpType.is_equal,
                            op1=mybir.AluOpType.mult,
                            accum_out=tgtv_t[:, ivb, t, j:j + 1])
                # stream this block's partials out on the near-idle gpsimd
                # ring, overlapped with the next block's compute
                nc.gpsimd.dma_start(negm.ap()[:, ivb], negm_t[:, ivb])
                nc.gpsimd.dma_start(ssum.ap()[:, ivb], ssum_t[:, ivb])
                nc.gpsimd.dma_start(tgtv.ap()[:, ivb], tgtv_t[:, ivb])

    nc.compile()
    return nc


def _get_compiled():
    key = "full"
    if key not in _COMPILED:
        _COMPILED[key] = build_nc()
    return _COMPILED[key]


def _prep_inputs(hidden_states, targets, weight_matrix):
    """Host-side shard + layout prep. Returns per-core in_maps."""
    h = np.ascontiguousarray(np.asarray(hidden_states, dtype=np.float32)
                             ).reshape(N_TOK, D)
    tgt = np.asarray(targets).reshape(N_TOK).astype(np.int64)
    W = np.asarray(weight_matrix, dtype=np.float32)

    mm_np = FP8 if USE_FP8 else BF16
    # h blocked: [t, p(d within k-tile), k, m(token within tile)]
    hb = np.ascontiguousarray(
        h.astype(mm_np).reshape(N_T, P, N_K, P).transpose(0, 3, 2, 1))

    iota = np.ascontiguousarray(
        np.broadcast_to(np.arange(VB, dtype=np.float32), (P, VB)))

    tl = tgt.reshape(N_T, P)  # [t, p]
    vb_off = (np.arange(N_VB, dtype=np.int64) * VB)[None, :, None]

    in_maps = []
    for c in range(NCORES):
        Wc = W[c * VSHARD:(c + 1) * VSHARD]
        wb = np.ascontiguousarray(
            Wc.astype(mm_np).reshape(N_VB, VB, N_K, P).transpose(3, 0, 2, 1))
        tgl = (tl.T[:, None, :] - c * VSHARD - vb_off).astype(np.float32)
        in_maps.append({"wt": wb, "ht": hb, "tg": np.ascontiguousarray(tgl),
                        "io": iota})
    return in_maps


def _combine(results):
    """float64 logsumexp-merge of per-core per-tile partials -> scalar loss."""
    m = np.stack([-r["negm"].astype(np.float64) for r in results])  # [C,P,vb,t,j]
    s = np.stack([r["ssum"].astype(np.float64) for r in results])
    tv = np.stack([r["tgtv"].astype(np.float64) for r in results])

    # partial axes: core, vb, j -> merge per (p, t)
    m2 = m.transpose(1, 3, 0, 2, 4).reshape(P, N_T, -1)   # [p, t, parts]
    s2 = s.transpose(1, 3, 0, 2, 4).reshape(P, N_T, -1)
    M = m2.max(axis=-1)                                    # [p, t]
    Ssum = (s2 * np.exp(m2 - M[..., None])).sum(axis=-1)
    lse = M + np.log(Ssum)                                 # [p, t]

    tgt_logit = tv.sum(axis=(0, 2, 4))                     # [p, t]
    loss = float((lse - tgt_logit).mean())
    return np.array(loss, dtype=np.float32)


def kernel(hidden_states, targets, weight_matrix):
    nc = _get_compiled()
    in_maps = _prep_inputs(hidden_states, targets, weight_matrix)
    res = run_bass_kernel_spmd(nc, in_maps, core_ids=list(range(NCORES)))
    return _combine(res.results)
